# revision 2
# baseline (speedup 1.0000x reference)
"""Trainium2 Bass kernel for DLLinearZeroDiagonal:
    y = x @ W.T + bias,  W = zero-diagonal 4096x4096 with strict triangles
    packed row-major in upper_w / lower_w.

Strategy (8 NeuronCores):
  - 2-way shard over output dim (o) x 4-way shard over batch (b).
  - Host reconstructs the dense weight (sanctioned by the sharding hint:
    "replicate the reconstructed weight") and lays out W^T / x^T shards in
    the tile order the device DMAs want.  All FLOPs + bias happen on device.
  - Per core: resident x^T shard (16 MB SBUF), stream W^T slabs once,
    1024 accumulating fp32r matmuls (128x128 @ 128x512), bias add on DVE,
    outputs written as y^T shard and untransposed on host.
"""

import numpy as np

N = 4096            # in/out feature dim and batch
RO, RB = 2, 4       # shard ways over output-dim / batch
OC = N // RO        # 2048 output cols per core
BC = N // RB        # 1024 batch rows per core
NW = OC // 128      # 16 stationary o-blocks per core
NT = N // 128       # 32 contraction tiles
NN = BC // 512      # 2 moving b-tiles per core

_PROGRAM = None


def _build_program():
    import concourse.bacc as bacc
    import concourse.bass as bass
    import concourse.tile as tile
    from concourse import mybir
    from contextlib import ExitStack

    F32 = mybir.dt.float32
    F32R = mybir.dt.float32r

    nc = bacc.Bacc("TRN2", target_bir_lowering=False, debug=False)
    # host-tiled layouts (see _shard_inputs):
    #   xt[t, p, b]     = x[b0+b, 128t+p]
    #   wt[w, p, t, o'] = W[o0+128w+o', 128t+p]
    #   bias2[p, w]     = bias[o0+128w+p]
    xt = nc.dram_tensor("xt", [NT, 128, BC], F32R, kind="ExternalInput")
    wt = nc.dram_tensor("wt", [NW, 128, NT, 128], F32R, kind="ExternalInput")
    bias = nc.dram_tensor("bias", [128, NW], F32, kind="ExternalInput")
    yt = nc.dram_tensor("yt", [OC, BC], F32, kind="ExternalOutput")

    with tile.TileContext(nc) as tc, ExitStack() as ctx:
        xtp = ctx.enter_context(tc.tile_pool(name="xtp", bufs=1))
        wtp = ctx.enter_context(tc.tile_pool(name="wtp", bufs=2))
        bp = ctx.enter_context(tc.tile_pool(name="bp", bufs=1))
        op = ctx.enter_context(tc.tile_pool(name="op", bufs=4))
        pp = ctx.enter_context(tc.tile_pool(name="pp", bufs=8, space="PSUM"))

        # resident x^T shard: [128, NT*BC] ; column block t holds j=128t+p
        xt_res = xtp.tile([128, NT * BC], F32R)
        for t in range(NT):
            nc.sync.dma_start(
                xt_res[:, t * BC:(t + 1) * BC],
                bass.AP(xt, t * 128 * BC, [[BC, 128], [1, BC]]),
            )
        bias_sb = bp.tile([128, NW], F32)
        nc.sync.dma_start(bias_sb[:], bass.AP(bias, 0, [[NW, 128], [1, NW]]))

        for w in range(NW):
            # stationary slab for o-block w: [128 (j in t), NT*128 (t, o')]
            slab = wtp.tile([128, NT * 128], F32R)
            nc.sync.dma_start(
                slab[:],
                bass.AP(wt, w * 128 * NT * 128, [[NT * 128, 128], [1, NT * 128]]),
            )
            psums = [pp.tile([128, 512], F32, name=f"ps{n}", tag="ps")
                     for n in range(NN)]
            for t in range(NT):
                lhsT = slab[:, t * 128:(t + 1) * 128]
                for n in range(NN):
                    nc.tensor.matmul(
                        psums[n][:],
                        lhsT,
                        xt_res[:, t * BC + n * 512: t * BC + n * 512 + 512],
                        start=(t == 0),
                        stop=(t == NT - 1),
                    )
            for n in range(NN):
                ot = op.tile([128, 512], F32)
                nc.vector.tensor_scalar_add(ot[:], psums[n][:], bias_sb[:, w:w + 1])
                nc.scalar.dma_start(
                    bass.AP(yt, w * 128 * BC + n * 512, [[BC, 128], [1, 512]]),
                    ot[:],
                )
    nc.compile()
    return nc


def _get_program():
    global _PROGRAM
    if _PROGRAM is None:
        _PROGRAM = _build_program()
    return _PROGRAM


def _reconstruct_wt(upper_w: np.ndarray, lower_w: np.ndarray) -> np.ndarray:
    """Dense W [o, j] from the packed strict triangles (row-major fill)."""
    W = np.zeros((N, N), dtype=np.float32)
    iu = np.triu_indices(N, k=1)
    il = np.tril_indices(N, k=-1)
    W[iu] = upper_w
    W[il] = lower_w
    return W


def _shard_inputs(x, upper_w, lower_w, bias):
    x = np.asarray(x, dtype=np.float32)
    upper_w = np.asarray(upper_w, dtype=np.float32)
    lower_w = np.asarray(lower_w, dtype=np.float32)
    bias = np.asarray(bias, dtype=np.float32)

    W = _reconstruct_wt(upper_w, lower_w)

    wt_shards = []
    bias_shards = []
    for ob in range(RO):
        Ws = W[ob * OC:(ob + 1) * OC, :]                       # [OC o, N j]
        # wt[w, p, t, o'] = Ws[128w+o', 128t+p]
        wt = np.ascontiguousarray(
            Ws.T.reshape(NT, 128, NW, 128).transpose(2, 1, 0, 3)
        )
        wt_shards.append(wt)
        bias_shards.append(
            np.ascontiguousarray(bias[ob * OC:(ob + 1) * OC].reshape(NW, 128).T)
        )

    xt_shards = []
    for bb in range(RB):
        xs = x[bb * BC:(bb + 1) * BC, :]                       # [BC b, N j]
        xt_shards.append(np.ascontiguousarray(xs.T.reshape(NT, 128, BC)))

    in_maps = []
    for c in range(8):
        ob, bb = c // RB, c % RB
        in_maps.append({
            "xt": xt_shards[bb],
            "wt": wt_shards[ob],
            "bias": bias_shards[ob],
        })
    return in_maps


def _assemble(results) -> np.ndarray:
    y = np.empty((N, N), dtype=np.float32)
    for c in range(8):
        ob, bb = c // RB, c % RB
        y[bb * BC:(bb + 1) * BC, ob * OC:(ob + 1) * OC] = results[c]["yt"].T
    return y


def kernel(x, upper_w, lower_w, bias):
    from concourse import bass_utils

    nc = _get_program()
    in_maps = _shard_inputs(x, upper_w, lower_w, bias)
    res = bass_utils.run_bass_kernel_spmd(nc, in_maps, core_ids=list(range(8)))
    return _assemble(res.results)


# revision 3
# speedup vs baseline: 116.5345x; 116.5345x over previous
"""Trainium2 Bass kernel for DLLinearZeroDiagonal:
    y = x @ W.T + bias,  W = zero-diagonal 4096x4096 with strict triangles
    packed row-major in upper_w / lower_w.

Strategy (8 NeuronCores):
  - 2-way shard over output dim (o) x 4-way shard over batch (b).
  - Host reconstructs the dense weight (sanctioned by the sharding hint:
    "replicate the reconstructed weight") and lays out W^T / x^T shards in
    the tile order the device DMAs want.  All FLOPs + bias happen on device.
  - Per core: resident x^T shard (16 MB SBUF), stream W^T slabs once,
    1024 accumulating fp32r matmuls (128x128 @ 128x512), bias add on DVE,
    outputs written as y^T shard and untransposed on host.
"""

import numpy as np

N = 4096            # in/out feature dim and batch
RO, RB = 2, 4       # shard ways over output-dim / batch
OC = N // RO        # 2048 output cols per core
BC = N // RB        # 1024 batch rows per core
NW = OC // 128      # 16 stationary o-blocks per core
NT = N // 128       # 32 contraction tiles
NN = BC // 512      # 2 moving b-tiles per core

_PROGRAM = None


def _build_program(reps=None):
    import concourse.bacc as bacc
    import concourse.bass as bass
    import concourse.tile as tile
    from concourse import mybir
    from contextlib import ExitStack, nullcontext

    F32 = mybir.dt.float32
    F32R = mybir.dt.float32r

    nc = bacc.Bacc("TRN2", target_bir_lowering=False, debug=False)
    # host-tiled layouts (see _shard_inputs):
    #   xt[t, p, b]     = x[b0+b, 128t+p]
    #   wt[w, p, t, o'] = W[o0+128w+o', 128t+p]
    #   bias2[p, w]     = bias[o0+128w+p]
    xt = nc.dram_tensor("xt", [NT, 128, BC], F32R, kind="ExternalInput")
    wt = nc.dram_tensor("wt", [NW, 128, NT, 128], F32R, kind="ExternalInput")
    bias = nc.dram_tensor("bias", [128, NW], F32, kind="ExternalInput")
    yt = nc.dram_tensor("yt", [OC, BC], F32, kind="ExternalOutput")

    with tile.TileContext(nc) as tc, ExitStack() as ctx:
        xtp = ctx.enter_context(tc.tile_pool(name="xtp", bufs=1))
        wtp = ctx.enter_context(tc.tile_pool(name="wtp", bufs=2))
        bp = ctx.enter_context(tc.tile_pool(name="bp", bufs=1))
        op = ctx.enter_context(tc.tile_pool(name="op", bufs=4))
        pp = ctx.enter_context(tc.tile_pool(name="pp", bufs=8, space="PSUM"))

        loop = tc.For_i(0, reps, 1) if reps is not None else nullcontext()
        with loop:
            # resident x^T shard: [128, NT*BC] ; column block t holds j=128t+p
            xt_res = xtp.tile([128, NT * BC], F32R)
            for t in range(NT):
                nc.sync.dma_start(
                    xt_res[:, t * BC:(t + 1) * BC],
                    bass.AP(xt, t * 128 * BC, [[BC, 128], [1, BC]]),
                )
            bias_sb = bp.tile([128, NW], F32)
            nc.sync.dma_start(bias_sb[:], bass.AP(bias, 0, [[NW, 128], [1, NW]]))

            for w in range(NW):
                # stationary slab for o-block w: [128 (j in t), NT*128 (t, o')]
                slab = wtp.tile([128, NT * 128], F32R)
                nc.sync.dma_start(
                    slab[:],
                    bass.AP(wt, w * 128 * NT * 128,
                            [[NT * 128, 128], [1, NT * 128]]),
                )
                psums = [pp.tile([128, 512], F32, name=f"ps{n}", tag="ps")
                         for n in range(NN)]
                for t in range(NT):
                    lhsT = slab[:, t * 128:(t + 1) * 128]
                    for n in range(NN):
                        nc.tensor.matmul(
                            psums[n][:],
                            lhsT,
                            xt_res[:, t * BC + n * 512: t * BC + n * 512 + 512],
                            start=(t == 0),
                            stop=(t == NT - 1),
                        )
                for n in range(NN):
                    ot = op.tile([128, 512], F32)
                    nc.vector.tensor_scalar_add(ot[:], psums[n][:],
                                                bias_sb[:, w:w + 1])
                    nc.scalar.dma_start(
                        bass.AP(yt, w * 128 * BC + n * 512, [[BC, 128], [1, 512]]),
                        ot[:],
                    )
    nc.compile()
    return nc


def _get_program():
    global _PROGRAM
    if _PROGRAM is None:
        _PROGRAM = _build_program()
    return _PROGRAM


def _reconstruct_wt(upper_w: np.ndarray, lower_w: np.ndarray) -> np.ndarray:
    """Dense W [o, j] from the packed strict triangles (row-major fill)."""
    W = np.zeros((N, N), dtype=np.float32)
    iu = np.triu_indices(N, k=1)
    il = np.tril_indices(N, k=-1)
    W[iu] = upper_w
    W[il] = lower_w
    return W


def _shard_inputs(x, upper_w, lower_w, bias):
    x = np.asarray(x, dtype=np.float32)
    upper_w = np.asarray(upper_w, dtype=np.float32)
    lower_w = np.asarray(lower_w, dtype=np.float32)
    bias = np.asarray(bias, dtype=np.float32)

    W = _reconstruct_wt(upper_w, lower_w)

    wt_shards = []
    bias_shards = []
    for ob in range(RO):
        Ws = W[ob * OC:(ob + 1) * OC, :]                       # [OC o, N j]
        # wt[w, p, t, o'] = Ws[128w+o', 128t+p]
        wt = np.ascontiguousarray(
            Ws.T.reshape(NT, 128, NW, 128).transpose(2, 1, 0, 3)
        )
        wt_shards.append(wt)
        bias_shards.append(
            np.ascontiguousarray(bias[ob * OC:(ob + 1) * OC].reshape(NW, 128).T)
        )

    xt_shards = []
    for bb in range(RB):
        xs = x[bb * BC:(bb + 1) * BC, :]                       # [BC b, N j]
        xt_shards.append(np.ascontiguousarray(xs.T.reshape(NT, 128, BC)))

    in_maps = []
    for c in range(8):
        ob, bb = c // RB, c % RB
        in_maps.append({
            "xt": xt_shards[bb],
            "wt": wt_shards[ob],
            "bias": bias_shards[ob],
        })
    return in_maps


def _assemble(results) -> np.ndarray:
    y = np.empty((N, N), dtype=np.float32)
    for c in range(8):
        ob, bb = c // RB, c % RB
        y[bb * BC:(bb + 1) * BC, ob * OC:(ob + 1) * OC] = results[c]["yt"].T
    return y


def kernel(x, upper_w, lower_w, bias):
    from concourse import bass_utils

    nc = _get_program()
    in_maps = _shard_inputs(x, upper_w, lower_w, bias)
    res = bass_utils.run_bass_kernel_spmd(nc, in_maps, core_ids=list(range(8)))
    return _assemble(res.results)


# revision 5
# speedup vs baseline: 233.1873x; 2.0010x over previous
"""Trainium2 Bass kernel for DLLinearZeroDiagonal:
    y = x @ W.T + bias,  W = zero-diagonal 4096x4096 with strict triangles
    packed row-major in upper_w / lower_w.

Strategy (8 NeuronCores):
  - 2-way shard over output dim (o) x 4-way shard over batch (b).
  - Host reconstructs the dense weight (sanctioned by the sharding hint:
    "replicate the reconstructed weight") and lays out W^T / x^T shards in
    the tile order the device DMAs want.  All FLOPs + bias happen on device.
  - Per core: resident x^T shard (16 MB SBUF), stream W^T slabs once,
    1024 accumulating fp32r matmuls (128x128 @ 128x512), bias add on DVE,
    outputs written as y^T shard and untransposed on host.
"""

import numpy as np

N = 4096            # in/out feature dim and batch
RO, RB = 2, 4       # shard ways over output-dim / batch
OC = N // RO        # 2048 output cols per core
BC = N // RB        # 1024 batch rows per core
NW = OC // 128      # 16 stationary o-blocks per core
NT = N // 128       # 32 contraction tiles
NN = BC // 512      # 2 moving b-tiles per core

_PROGRAM = None


def _build_program(reps=None):
    import concourse.bacc as bacc
    import concourse.bass as bass
    import concourse.tile as tile
    from concourse import mybir
    from contextlib import ExitStack, nullcontext

    F32 = mybir.dt.float32
    F32R = mybir.dt.float32r

    nc = bacc.Bacc("TRN2", target_bir_lowering=False, debug=False)
    # host-tiled layouts (see _shard_inputs):
    #   xt[t, p, b]     = x[b0+b, 128t+p]
    #   wt[w, p, t, o'] = W[o0+128w+o', 128t+p]
    #   bias2[p, w]     = bias[o0+128w+p]
    xt = nc.dram_tensor("xt", [NT, 128, BC], F32R, kind="ExternalInput")
    wt = nc.dram_tensor("wt", [NW, 128, NT, 128], F32R, kind="ExternalInput")
    bias = nc.dram_tensor("bias", [128, NW], F32, kind="ExternalInput")
    yt = nc.dram_tensor("yt", [OC, BC], F32, kind="ExternalOutput")

    with tile.TileContext(nc) as tc, ExitStack() as ctx:
        xtp = ctx.enter_context(tc.tile_pool(name="xtp", bufs=1))
        wtp = ctx.enter_context(tc.tile_pool(name="wtp", bufs=2))
        bp = ctx.enter_context(tc.tile_pool(name="bp", bufs=1))
        op = ctx.enter_context(tc.tile_pool(name="op", bufs=4))
        pp = ctx.enter_context(tc.tile_pool(name="pp", bufs=8, space="PSUM"))

        loop = tc.For_i(0, reps, 1) if reps is not None else nullcontext()
        with loop:
            # resident x^T shard: [128, NT*BC] ; column block t holds j=128t+p
            xt_res = xtp.tile([128, NT * BC], F32R)
            for t in range(NT):
                nc.sync.dma_start(
                    xt_res[:, t * BC:(t + 1) * BC],
                    bass.AP(xt, t * 128 * BC, [[BC, 128], [1, BC]]),
                )
            bias_sb = bp.tile([128, NW], F32)
            nc.sync.dma_start(bias_sb[:], bass.AP(bias, 0, [[NW, 128], [1, NW]]))

            for w in range(NW):
                # stationary slab for o-block w: [128 (j in t), NT*128 (t, o')]
                slab = wtp.tile([128, NT * 128], F32R)
                nc.sync.dma_start(
                    slab[:],
                    bass.AP(wt, w * 128 * NT * 128,
                            [[NT * 128, 128], [1, NT * 128]]),
                )
                psums = [pp.tile([128, 512], F32, name=f"ps{n}", tag="ps")
                         for n in range(NN)]
                for t in range(NT):
                    lhsT = slab[:, t * 128:(t + 1) * 128]
                    for n in range(NN):
                        nc.tensor.matmul(
                            psums[n][:],
                            lhsT,
                            xt_res[:, t * BC + n * 512: t * BC + n * 512 + 512],
                            start=(t == 0),
                            stop=(t == NT - 1),
                        )
                for n in range(NN):
                    ot = op.tile([128, 512], F32)
                    nc.vector.tensor_scalar_add(ot[:], psums[n][:],
                                                bias_sb[:, w:w + 1])
                    nc.scalar.dma_start(
                        bass.AP(yt, w * 128 * BC + n * 512, [[BC, 128], [1, 512]]),
                        ot[:],
                    )
    nc.compile()
    return nc


def _get_program():
    global _PROGRAM
    if _PROGRAM is None:
        _PROGRAM = _build_program()
    return _PROGRAM


def _reconstruct_wt(upper_w: np.ndarray, lower_w: np.ndarray) -> np.ndarray:
    """Dense W [o, j] from the packed strict triangles (row-major fill)."""
    W = np.zeros((N, N), dtype=np.float32)
    iu = np.triu_indices(N, k=1)
    il = np.tril_indices(N, k=-1)
    W[iu] = upper_w
    W[il] = lower_w
    return W


def _shard_inputs(x, upper_w, lower_w, bias):
    x = np.asarray(x, dtype=np.float32)
    upper_w = np.asarray(upper_w, dtype=np.float32)
    lower_w = np.asarray(lower_w, dtype=np.float32)
    bias = np.asarray(bias, dtype=np.float32)

    W = _reconstruct_wt(upper_w, lower_w)

    wt_shards = []
    bias_shards = []
    for ob in range(RO):
        Ws = W[ob * OC:(ob + 1) * OC, :]                       # [OC o, N j]
        # wt[w, p, t, o'] = Ws[128w+o', 128t+p]
        wt = np.ascontiguousarray(
            Ws.T.reshape(NT, 128, NW, 128).transpose(2, 1, 0, 3)
        )
        wt_shards.append(wt)
        bias_shards.append(
            np.ascontiguousarray(bias[ob * OC:(ob + 1) * OC].reshape(NW, 128).T)
        )

    xt_shards = []
    for bb in range(RB):
        xs = x[bb * BC:(bb + 1) * BC, :]                       # [BC b, N j]
        xt_shards.append(np.ascontiguousarray(xs.T.reshape(NT, 128, BC)))

    in_maps = []
    for c in range(8):
        ob, bb = c // RB, c % RB
        in_maps.append({
            "xt": xt_shards[bb],
            "wt": wt_shards[ob],
            "bias": bias_shards[ob],
        })
    return in_maps


def _assemble(results) -> np.ndarray:
    y = np.empty((N, N), dtype=np.float32)
    for c in range(8):
        ob, bb = c // RB, c % RB
        y[bb * BC:(bb + 1) * BC, ob * OC:(ob + 1) * OC] = results[c]["yt"].T
    return y


def kernel(x, upper_w, lower_w, bias):
    from concourse import bass_utils

    nc = _get_program()
    in_maps = _shard_inputs(x, upper_w, lower_w, bias)
    res = bass_utils.run_bass_kernel_spmd(nc, in_maps, core_ids=list(range(8)))
    return _assemble(res.results)
